# revision 1
# baseline (speedup 1.0000x reference)
"""CrossSharedUnit Trainium2 kernel — 8-core data-parallel over batch.

Reference computation (per batch b, S=128 tokens, H=512 hidden, K=8):
  proj[b,s,k,g] = sum_h left[b,s,h] * G[h,k,g]
  raw[b,s,t,k]  = tanh(sum_g proj[b,s,k,g] * right[b,t,g])
  score[b,s,t]  = sum_k raw[b,s,t,k] * v[k]
  attn          = softmax(score, axis=t)
  out           = self + attn @ other_hidden
for two branches (aspect: left=aspect, right=polarity; polarity: left=aspect,
right=aspect — faithful to the source which uses aspect on BOTH sides).

Sharding: batch B=32 split 4-per-core across 8 cores; G tensors replicated.
No collectives.

Schedule: the PE (tensor engine) is the bottleneck (~78us of fp32r matmul),
so the program is one continuous PE stream:
  warmup | br0-s1 (+ br0-s2-ck0 spliced at k=3) | br0-s2-ck1
         | br1-s1 (+ br1-s2-ck0 splice + br0 z/out mms interleaved)
         | br1-s2-ck1 | br1 z/out
with softmax chains on vector/gpsimd/scalar underneath the next phase's
matmuls. All input DMA issues live on the sync queue in exact consumption
order (plus two tiny v-loads on gpsimd), so a blocked G prefetch (bufs=1
buffer recycling between branches) can never head-of-line-block a PSUM
evacuation; output stores ride the sync queue after the loads are done.

All matmuls are float32r (TF32-like, 1 row/cycle at free>=256). Softmax
needs no max-subtraction: |score| <= sum|v_k| so exp() cannot overflow in
fp32. The softmax division is deferred through the attention matmul:
out = self + (E @ other) / Z with Z from a ones-matmul.
"""

import os
import sys

sys.path.insert(0, "/opt/trn_rl_repo")

import numpy as np

from concourse import bacc, mybir, tile
from concourse.bass_utils import run_bass_kernel_spmd

B, S, H, K = 32, 128, 512, 8
NCORES = 8
BL = B // NCORES          # batches per core
BS = BL * S               # rows per core (512)
P = 128                   # partitions
HT = H // P               # h partition-tiles (4)
KG = K * H                # flattened (k,g) axis (4096)
KC = K // 2               # k's per stage-2 chunk (4)
F32 = mybir.dt.float32
F32R = mybir.dt.float32r

_cache = {}


def _build():
    """Build + compile the per-core Bass program (same program on all cores)."""
    nc = bacc.Bacc("TRN2", target_bir_lowering=False, debug=False,
                   num_devices=NCORES)

    xa_t_d = nc.dram_tensor("xa_t", [P, HT * BS], F32R, kind="ExternalInput")
    xp_t_d = nc.dram_tensor("xp_t", [P, HT * BS], F32R, kind="ExternalInput")
    xa_nat_d = nc.dram_tensor("xa_nat", [P, BL * H], F32R, kind="ExternalInput")
    xp_nat_d = nc.dram_tensor("xp_nat", [P, BL * H], F32R, kind="ExternalInput")
    # G: one pre-shuffled tensor per branch; pieces are column slices in
    # consumption order: 4x k0 per-h, 2x k1 halves, 6x per-k
    g_ap_d = nc.dram_tensor("g_ap", [P, HT * KG], F32R, kind="ExternalInput")
    g_pa_d = nc.dram_tensor("g_pa", [P, HT * KG], F32R, kind="ExternalInput")
    v_ap_d = nc.dram_tensor("v_ap", [K, 1], F32, kind="ExternalInput")
    v_pa_d = nc.dram_tensor("v_pa", [K, 1], F32, kind="ExternalInput")
    out_a_d = nc.dram_tensor("out_a", [BS, H], F32, kind="ExternalOutput")
    out_p_d = nc.dram_tensor("out_p", [BS, H], F32, kind="ExternalOutput")

    Tanh = mybir.ActivationFunctionType.Tanh
    Exp = mybir.ActivationFunctionType.Exp
    MULT = mybir.AluOpType.mult
    ADD = mybir.AluOpType.add

    with tile.TileContext(nc) as tc:
        with (
            tc.tile_pool(name="const", bufs=1) as cpool,
            tc.tile_pool(name="g", bufs=1) as gpool,
            tc.tile_pool(name="proj", bufs=1) as projpool,
            tc.tile_pool(name="work", bufs=2) as work,
            tc.tile_pool(name="ps_acc", bufs=4, space="PSUM") as ps_acc,
            tc.tile_pool(name="ps_o", bufs=2, space="PSUM") as ps_o,
            tc.tile_pool(name="ps_z", bufs=2, space="PSUM") as ps_z,
        ):
            # ---- constants + warmup weights (vector queue) --------------
            wm = cpool.tile([P, BS], F32R, tag="wm")
            nc.vector.memset(wm[:].bitcast(F32), 0.0)
            ones_t = cpool.tile([P, 2], F32R, tag="ones_t")
            nc.vector.memset(ones_t[:].bitcast(F32), 1.0)

            # ---- persistent activations --------------------------------
            xa_t0 = cpool.tile([P, BS], F32R, tag="xa_t0")
            xa_tb = cpool.tile([P, (HT - 1) * BS], F32R, tag="xa_tb")
            xp_t = cpool.tile([P, HT * BS], F32R, tag="xp_t")
            xa_nat = cpool.tile([P, BL * H], F32R, tag="xa_nat")
            xp_nat = cpool.tile([P, BL * H], F32R, tag="xp_nat")

            vrow_a = cpool.tile([1, K], F32, tag="vrow_a")
            vrow_p = cpool.tile([1, K], F32, tag="vrow_p")
            nc.gpsimd.dma_start(out=vrow_a[:], in_=v_ap_d.ap().rearrange("k o -> o k"))
            nc.gpsimd.dma_start(out=vrow_p[:], in_=v_pa_d.ap().rearrange("k o -> o k"))
            vbc_a = cpool.tile([P, K], F32, tag="vbc_a")
            vbc_p = cpool.tile([P, K], F32, tag="vbc_p")
            nc.gpsimd.partition_broadcast(vbc_a[:], vrow_a[:])
            nc.gpsimd.partition_broadcast(vbc_p[:], vrow_p[:])

            # ---- G piece tiles (shared bufs=1 between branches) ---------
            g_k0 = [gpool.tile([P, H], F32R, tag=f"g_k0_{h}",
                                name=f"g_k0_{h}") for h in range(HT)]
            g_k1a = gpool.tile([P, 2 * H], F32R, tag="g_k1a")
            g_k1b = gpool.tile([P, 2 * H], F32R, tag="g_k1b")
            g_kk = [gpool.tile([P, HT * H], F32R, tag=f"g_kk{i}",
                               name=f"g_kk{i}")
                     for i in range(6)]

            # ---- the loader. Host arrays are pre-shuffled partition-major
            # so every DMA is 128 descriptors of one big contiguous chunk.
            # The startup-critical pieces (k0 G + aspect) are 256KB each and
            # spread over three queues for DMA-engine concurrency.
            def load_g_head(g_d):
                for h in range(HT):
                    nc.sync.dma_start(out=g_k0[h][:],
                                      in_=g_d.ap()[:, h * H:(h + 1) * H])
                nc.sync.dma_start(out=g_k1a[:],
                                  in_=g_d.ap()[:, HT * H:HT * H + 2 * H])
                nc.sync.dma_start(out=g_k1b[:],
                                  in_=g_d.ap()[:, HT * H + 2 * H:2 * HT * H])

            def load_g_k(g_d, i):
                o = (2 + i) * HT * H
                nc.sync.dma_start(out=g_kk[i][:], in_=g_d.ap()[:, o:o + HT * H])

            # interleave xa-h / G-k0-h so the PE can start after 512KB and
            # gets one h-step per ~0.8us thereafter
            nc.sync.dma_start(out=xa_t0[:], in_=xa_t_d.ap()[:, 0:BS])
            nc.sync.dma_start(out=g_k0[0][:], in_=g_ap_d.ap()[:, 0:H])
            for h in range(1, HT):
                nc.sync.dma_start(out=xa_tb[:, (h - 1) * BS:h * BS],
                                  in_=xa_t_d.ap()[:, h * BS:(h + 1) * BS])
                nc.sync.dma_start(out=g_k0[h][:],
                                  in_=g_ap_d.ap()[:, h * H:(h + 1) * H])
            nc.sync.dma_start(out=g_k1a[:],
                              in_=g_ap_d.ap()[:, HT * H:HT * H + 2 * H])
            nc.sync.dma_start(out=g_k1b[:],
                              in_=g_ap_d.ap()[:, HT * H + 2 * H:2 * HT * H])
            for i in range(4):
                load_g_k(g_ap_d, i)
            nc.sync.dma_start(out=xp_t[:], in_=xp_t_d.ap()[:])
            load_g_k(g_ap_d, 4)
            load_g_k(g_ap_d, 5)
            nc.sync.dma_start(out=xp_nat[:], in_=xp_nat_d.ap()[:])
            # (g_pa loads are emitted after br0-s1 so the WAR deps pick up
            #  br0's reads; xa_nat after those.)

            def g_lhsT(k, h, gt):
                if k == 0:
                    return g_k0[h][:, gt * P:(gt + 1) * P]
                if k == 1:
                    piece = g_k1a if h < 2 else g_k1b
                    o = (h % 2) * H + gt * P
                    return piece[:, o:o + P]
                piece = g_kk[k - 2]
                o = h * H + gt * P
                return piece[:, o:o + P]

            def xa_rhs(h):
                if h == 0:
                    return xa_t0[:]
                return xa_tb[:, (h - 1) * BS:h * BS]

            def xa_lhsT(gi, b):
                if gi == 0:
                    return xa_t0[:, b * S:(b + 1) * S]
                o = (gi - 1) * BS + b * S
                return xa_tb[:, o:o + S]

            def xp_lhsT(gi, b):
                o = gi * BS + b * S
                return xp_t[:, o:o + S]

            # projT2[gt][g_part, k, b, s] — stage-1 output, stage-2 rhs.
            projT2 = [projpool.tile([P, K, BL, S], F32R, tag=f"projT2_{gt}",
                                    name=f"projT2_{gt}")
                      for gt in range(HT)]

            evac_state = [0]

            def evac(dst, src):
                # ping-pong PSUM evacuations between vector and scalar
                if evac_state[0] % 2 == 0:
                    nc.vector.tensor_copy(dst, src)
                else:
                    nc.scalar.copy(dst, src)
                evac_state[0] += 1

            # ---- PE warmup: get the p-state ramp going during DMA lead-in
            for w in range(6):
                acc = ps_acc.tile([P, BL, S], F32, tag="acc", name=f"warm{w}")
                nc.tensor.matmul(acc[:], wm[:, 0:P], wm[:],
                                 start=True, stop=True)

            def stage1(br):
                # k0 h-outer with 4 open accumulators: first matmuls need
                # only g_k0h0 + xa_t0 (512KB total).
                accs = [ps_acc.tile([P, BL, S], F32, tag="acc",
                                    name=f"s1a{br}k0g{gt}")
                        for gt in range(HT)]
                for h in range(HT):
                    for gt in range(HT):
                        nc.tensor.matmul(
                            accs[gt][:], g_lhsT(0, h, gt), xa_rhs(h),
                            start=(h == 0), stop=(h == HT - 1),
                            skip_group_check=True)
                for gt in range(HT):
                    evac(projT2[gt][:, 0, :, :], accs[gt][:])
                for k in range(1, K):
                    for gt in range(HT):
                        acc = ps_acc.tile([P, BL, S], F32, tag="acc",
                                          name=f"s1a{br}k{k}g{gt}")
                        for h in range(HT):
                            nc.tensor.matmul(
                                acc[:], g_lhsT(k, h, gt), xa_rhs(h),
                                start=(h == 0), stop=(h == HT - 1))
                        evac(projT2[gt][:, k, :, :], acc[:])
                    yield k

            # th_all[t_part, k, b, s]: tanh(stage-2) output, both branches
            # (WAR-recycled). Score ops slice [:, j, :, :] batched over b.
            th_all = work.tile([P, K, BL, S], F32, tag="th", bufs=1)

            def stage2_ck(br, lhsT_of, ck):
                # raw[t, k, s] = tanh(sum_g right[t,g] proj[g,k,s]) per batch
                for b in range(BL):
                    acc2 = ps_acc.tile([P, KC, S], F32, tag="acc",
                                       name=f"s2a{br}b{b}c{ck}")
                    for gi in range(HT):
                        nc.tensor.matmul(
                            acc2[:],
                            lhsT_of(gi, b),
                            projT2[gi][:, ck * KC:(ck + 1) * KC, b, :],
                            start=(gi == 0), stop=(gi == HT - 1))
                    nc.scalar.activation(
                        th_all[:, ck * KC:(ck + 1) * KC, b, :], acc2[:], Tanh)

            def sca_all(vbc):
                # first-half score partial, batched over all 4 batches
                sca = work.tile([P, BL, S], F32, tag="sca", bufs=1)
                nc.vector.tensor_scalar_mul(sca[:], th_all[:, 0, :, :],
                                            vbc[:, 0:1])
                for j in range(1, KC):
                    nc.vector.scalar_tensor_tensor(
                        sca[:], th_all[:, j, :, :], vbc[:, j:j + 1], sca[:],
                        MULT, ADD)
                return sca

            def zout(br, b, e_t, nat_other, nat_self, out_d):
                # out = self + (E_T.T @ other) / Z, Z via ones-matmul.
                zp = ps_z.tile([P, 2], F32, tag="z", name=f"z{br}b{b}")
                nc.tensor.matmul(zp[:], e_t[:, b, :], ones_t[:],
                                 start=True, stop=True)
                rz = work.tile([P, 1], F32, tag="rz", bufs=4)
                nc.vector.reciprocal(rz[:], zp[:, 0:1])
                rp = ps_o.tile([P, H], F32, tag="o", name=f"o{br}b{b}")
                nc.tensor.matmul(rp[:], e_t[:, b, :], nat_other[:, b * H:(b + 1) * H],
                                 start=True, stop=True)
                ot = work.tile([P, H], F32, tag="ot", bufs=2)
                nc.vector.scalar_tensor_tensor(
                    ot[:], rp[:], rz[:, 0:1], nat_self[:, b * H:(b + 1) * H].bitcast(F32),
                    MULT, ADD)
                nc.sync.dma_start(out=out_d.ap()[b * P:(b + 1) * P, :],
                                  in_=ot[:])

            e_t0 = work.tile([P, BL, S], F32R, tag="e0", bufs=1)
            e_t1 = work.tile([P, BL, S], F32R, tag="e1", bufs=1)

            # ================= branch 0 (aspect) ========================
            for k in stage1(0):
                if k == 5:
                    stage2_ck(0, xp_lhsT, 0)
                    sca0 = sca_all(vbc_a)
            load_g_head(g_pa_d)     # prefetch; WAR-gated on br0-s1 reads
            nc.sync.dma_start(out=xa_nat[:], in_=xa_nat_d.ap()[:])
            for i in range(6):
                load_g_k(g_pa_d, i)
            stage2_ck(0, xp_lhsT, 1)
            # batched second half + exp (runs under br1-s1)
            scb0 = work.tile([P, BL, S], F32, tag="scb", bufs=1)
            nc.vector.tensor_scalar_mul(scb0[:], th_all[:, KC, :, :],
                                        vbc_a[:, KC:KC + 1])
            for j in range(1, KC):
                nc.vector.scalar_tensor_tensor(
                    scb0[:], th_all[:, KC + j, :, :],
                    vbc_a[:, KC + j:KC + j + 1], scb0[:], MULT, ADD)
            nc.vector.tensor_tensor(scb0[:], sca0[:], scb0[:], ADD)
            nc.scalar.activation(e_t0[:], scb0[:], Exp)

            # ================= branch 1 (polarity) ======================
            # br1 stage 1 with br1-s2-ck0 spliced at k=3 and br0's z/out
            # matmuls interleaved so the PE never waits on softmax chains.
            zo = 0
            for k in stage1(1):
                if k == 5:
                    stage2_ck(1, xa_lhsT, 0)
                    sca1 = sca_all(vbc_p)
                elif k in (2, 3, 4, 6):
                    zout(0, zo, e_t0, xp_nat, xa_nat, out_a_d)
                    zo += 1
            stage2_ck(1, xa_lhsT, 1)

            # Tail: pair-batched second-half chains on vector; the
            # scale+residual combine rides scalar (act-Copy-scale) + gpsimd
            # (tensor add) so no engine saturates after the last tanh.
            Copy = mybir.ActivationFunctionType.Copy
            for pr in range(2):
                bs2 = slice(2 * pr, 2 * pr + 2)
                scb = work.tile([P, 2, S], F32, tag=f"scb1_{pr}", bufs=1)
                nc.vector.tensor_scalar_mul(scb[:], th_all[:, KC, bs2, :],
                                            vbc_p[:, KC:KC + 1])
                for j in range(1, KC):
                    nc.vector.scalar_tensor_tensor(
                        scb[:], th_all[:, KC + j, bs2, :],
                        vbc_p[:, KC + j:KC + j + 1], scb[:], MULT, ADD)
                nc.vector.tensor_tensor(scb[:], sca1[:, bs2, :], scb[:], ADD)
                nc.scalar.activation(e_t1[:, bs2, :], scb[:], Exp)
                for b in (2 * pr, 2 * pr + 1):
                    zp = ps_z.tile([P, 2], F32, tag="z", name=f"z1b{b}")
                    nc.tensor.matmul(zp[:], e_t1[:, b, :], ones_t[:],
                                     start=True, stop=True)
                    rz = work.tile([P, 1], F32, tag="rz", bufs=4)
                    nc.vector.reciprocal(rz[:], zp[:, 0:1])
                    rp = ps_o.tile([P, H], F32, tag="o", name=f"o1b{b}")
                    nc.tensor.matmul(rp[:], e_t1[:, b, :], xa_nat[:, b * H:(b + 1) * H],
                                     start=True, stop=True)
                    if pr == 0:
                        # early pair: combine on scalar+gpsimd, leaving
                        # vector free for the late pair's chain
                        ots = work.tile([P, H], F32, tag="ots", bufs=2)
                        nc.scalar.activation(ots[:], rp[:], Copy,
                                             scale=rz[:, 0:1])
                        otf = work.tile([P, H], F32, tag="otf", bufs=2)
                        nc.gpsimd.tensor_tensor(
                            otf[:], ots[:],
                            xp_nat[:, b * H:(b + 1) * H].bitcast(F32), ADD)
                    else:
                        # late pair: vector is idle by now — single stt
                        otf = work.tile([P, H], F32, tag="otf_v", bufs=2)
                        nc.vector.scalar_tensor_tensor(
                            otf[:], rp[:], rz[:, 0:1],
                            xp_nat[:, b * H:(b + 1) * H].bitcast(F32),
                            MULT, ADD)
                    nc.sync.dma_start(
                        out=out_p_d.ap()[b * P:(b + 1) * P, :], in_=otf[:])

    nc.compile()
    return nc


def _get_nc():
    if "nc" not in _cache:
        _cache["nc"] = _build()
    return _cache["nc"]


def _prep_in_maps(aspect_hidden, polarity_hidden, G_aspect_polarity,
                  G_polarity_aspect, G_vector_aspect, G_vector_polarity):
    f = np.float32

    def shuffle_g(g):
        # host-side image of the SBUF G piece tiles, concatenated in
        # consumption order: k0 per-h, k1, then per-k
        gr = np.asarray(g, dtype=f).reshape(HT, P, K, H)
        pieces = [gr[h, :, 0, :] for h in range(HT)]
        pieces.append(gr[:, :, 1, :].transpose(1, 0, 2).reshape(P, HT * H))
        for k in range(2, K):
            pieces.append(
                gr[:, :, k, :].transpose(1, 0, 2).reshape(P, HT * H))
        return np.ascontiguousarray(np.concatenate(pieces, axis=1))

    def shuffle_t(x_loc):
        # [BS,H] -> transposed partition-major [P, (ht, bs)]
        return np.ascontiguousarray(
            x_loc.T.reshape(HT, P, BS).transpose(1, 0, 2))

    def shuffle_nat(x_loc):
        # [BS,H] -> partition-major [P, (b, h)]
        return np.ascontiguousarray(
            x_loc.reshape(BL, P, H).transpose(1, 0, 2).reshape(P, BL * H))

    a = np.ascontiguousarray(aspect_hidden, dtype=f)
    p = np.ascontiguousarray(polarity_hidden, dtype=f)
    g_ap = shuffle_g(G_aspect_polarity)
    g_pa = shuffle_g(G_polarity_aspect)
    v_ap = np.ascontiguousarray(G_vector_aspect, dtype=f)
    v_pa = np.ascontiguousarray(G_vector_polarity, dtype=f)

    in_maps = []
    for c in range(NCORES):
        a_loc = a[c * BL:(c + 1) * BL].reshape(BS, H)
        p_loc = p[c * BL:(c + 1) * BL].reshape(BS, H)
        m = {
            "xa_t": np.ascontiguousarray(shuffle_t(a_loc).reshape(P, HT * BS)),
            "xp_t": np.ascontiguousarray(shuffle_t(p_loc).reshape(P, HT * BS)),
            "xa_nat": shuffle_nat(a_loc),
            "xp_nat": shuffle_nat(p_loc),
            "g_ap": g_ap,
            "g_pa": g_pa,
            "v_ap": v_ap,
            "v_pa": v_pa,
        }
        in_maps.append(m)
    return in_maps


def kernel(aspect_hidden, polarity_hidden, G_aspect_polarity,
           G_polarity_aspect, G_vector_aspect, G_vector_polarity):
    nc = _get_nc()
    in_maps = _prep_in_maps(aspect_hidden, polarity_hidden, G_aspect_polarity,
                            G_polarity_aspect, G_vector_aspect,
                            G_vector_polarity)
    res = run_bass_kernel_spmd(
        nc, in_maps, core_ids=list(range(NCORES)),
        trace=bool(os.environ.get("KERNEL_TRACE")))
    _cache["last_results"] = res

    out_a = np.empty((B, S, H), np.float32)
    out_p = np.empty((B, S, H), np.float32)
    for c in range(NCORES):
        out_a[c * BL:(c + 1) * BL] = res.results[c]["out_a"].reshape(BL, S, H)
        out_p[c * BL:(c + 1) * BL] = res.results[c]["out_p"].reshape(BL, S, H)
    return (out_a, out_p)



# revision 6
# speedup vs baseline: 1.0024x; 1.0024x over previous
"""CrossSharedUnit Trainium2 kernel — 8-core data-parallel over batch.

Reference computation (per batch b, S=128 tokens, H=512 hidden, K=8):
  proj[b,s,k,g] = sum_h left[b,s,h] * G[h,k,g]
  raw[b,s,t,k]  = tanh(sum_g proj[b,s,k,g] * right[b,t,g])
  score[b,s,t]  = sum_k raw[b,s,t,k] * v[k]
  attn          = softmax(score, axis=t)
  out           = self + attn @ other_hidden
for two branches (aspect: left=aspect, right=polarity; polarity: left=aspect,
right=aspect — faithful to the source which uses aspect on BOTH sides).

Sharding: batch B=32 split 4-per-core across 8 cores; G tensors replicated.
No collectives.

Schedule: the PE (tensor engine) is the bottleneck (~76us of fp32r matmul),
so the program is one continuous PE stream:
  warmup | br0-s1 (+ br0-s2-ck0 spliced at k=5) | br0-s2-ck1
         | br1-s1 (+ br1-s2-ck0 splice + br0 z/out mms interleaved)
         | br1-s2-ck1 (pairwise, with z/out interleaved per pair)
with softmax chains on vector/gpsimd/scalar underneath the next phase's
matmuls.

DMA: all input loads ride the sync queue in exact consumption order. Head
loads are >=4KB contiguous per partition row (the DMA engines aggregate
per-descriptor; 2KB rows ran at ~half throughput), so the startup stream
runs near peak and the PE never starves after the warmup ramp.

Tail: the last stage-2 chunk (k=4..7) is processed pair-of-batches at a
time; each pair's score chain is split across vector (k4,k5 + combine) and
gpsimd (k6,k7) so the exp is ~3 dependent ops after the tanh, and the
z/out matmuls for a pair ride the PE between the other pair's matmuls.

All matmuls are float32r (TF32-like, 1 row/cycle at free>=256). Softmax
needs no max-subtraction: |score| <= sum|v_k| so exp() cannot overflow in
fp32. The softmax division is deferred through the attention matmul:
out = self + (E @ other) / Z with Z from a ones-matmul.
"""

import os
import sys

sys.path.insert(0, "/opt/trn_rl_repo")

import numpy as np

from concourse import bacc, mybir, tile
from concourse.bass_utils import run_bass_kernel_spmd

B, S, H, K = 32, 128, 512, 8
NCORES = 8
BL = B // NCORES          # batches per core
BS = BL * S               # rows per core (512)
P = 128                   # partitions
HT = H // P               # h partition-tiles (4)
KG = K * H                # flattened (k,g) axis (4096)
KC = K // 2               # k's per stage-2 chunk (4)
F32 = mybir.dt.float32
F32R = mybir.dt.float32r

_cache = {}


def _build():
    """Build + compile the per-core Bass program (same program on all cores)."""
    nc = bacc.Bacc("TRN2", target_bir_lowering=False, debug=False,
                   num_devices=NCORES)

    xa_t_d = nc.dram_tensor("xa_t", [P, HT * BS], F32R, kind="ExternalInput")
    xp_t_d = nc.dram_tensor("xp_t", [P, HT * BS], F32R, kind="ExternalInput")
    xa_nat_d = nc.dram_tensor("xa_nat", [P, BL * H], F32R, kind="ExternalInput")
    xp_nat_d = nc.dram_tensor("xp_nat", [P, BL * H], F32R, kind="ExternalInput")
    # G: one pre-shuffled tensor per branch; column blocks in consumption
    # order: k0 per-h block (4H), k1 per-h block (4H), then per-k blocks
    g_ap_d = nc.dram_tensor("g_ap", [P, HT * KG], F32R, kind="ExternalInput")
    g_pa_d = nc.dram_tensor("g_pa", [P, HT * KG], F32R, kind="ExternalInput")
    v_ap_d = nc.dram_tensor("v_ap", [K, 1], F32, kind="ExternalInput")
    v_pa_d = nc.dram_tensor("v_pa", [K, 1], F32, kind="ExternalInput")
    out_a_d = nc.dram_tensor("out_a", [BS, H], F32, kind="ExternalOutput")
    out_p_d = nc.dram_tensor("out_p", [BS, H], F32, kind="ExternalOutput")

    Tanh = mybir.ActivationFunctionType.Tanh
    Exp = mybir.ActivationFunctionType.Exp
    Copy = mybir.ActivationFunctionType.Copy
    MULT = mybir.AluOpType.mult
    ADD = mybir.AluOpType.add

    with tile.TileContext(nc) as tc:
        with (
            tc.tile_pool(name="const", bufs=1) as cpool,
            tc.tile_pool(name="g", bufs=1) as gpool,
            tc.tile_pool(name="proj", bufs=1) as projpool,
            tc.tile_pool(name="work", bufs=2) as work,
            tc.tile_pool(name="ps_acc", bufs=4, space="PSUM") as ps_acc,
            tc.tile_pool(name="ps_o", bufs=2, space="PSUM") as ps_o,
            tc.tile_pool(name="ps_z", bufs=2, space="PSUM") as ps_z,
        ):
            # ---- constants + warmup weights --------------------------------
            wm = cpool.tile([P, BS], F32R, tag="wm")
            nc.vector.memset(wm[:].bitcast(F32), 0.0)
            ones_t = cpool.tile([P, 2], F32R, tag="ones_t")
            nc.vector.memset(ones_t[:].bitcast(F32), 1.0)

            # ---- persistent activations ------------------------------------
            xa_t = cpool.tile([P, HT * BS], F32R, tag="xa_t")
            xp_t = cpool.tile([P, HT * BS], F32R, tag="xp_t")
            xa_nat = cpool.tile([P, BL * H], F32R, tag="xa_nat")
            xp_nat = cpool.tile([P, BL * H], F32R, tag="xp_nat")

            vrow_a = cpool.tile([1, K], F32, tag="vrow_a")
            vrow_p = cpool.tile([1, K], F32, tag="vrow_p")
            nc.gpsimd.dma_start(out=vrow_a[:], in_=v_ap_d.ap().rearrange("k o -> o k"))
            nc.gpsimd.dma_start(out=vrow_p[:], in_=v_pa_d.ap().rearrange("k o -> o k"))
            vbc_a = cpool.tile([P, K], F32, tag="vbc_a")
            vbc_p = cpool.tile([P, K], F32, tag="vbc_p")
            nc.gpsimd.partition_broadcast(vbc_a[:], vrow_a[:])
            nc.gpsimd.partition_broadcast(vbc_p[:], vrow_p[:])

            # ---- G piece tiles (shared bufs=1 between branches) ------------
            g_k0 = gpool.tile([P, HT * H], F32R, tag="g_k0")
            g_k1 = gpool.tile([P, HT * H], F32R, tag="g_k1")
            g_kk = [gpool.tile([P, HT * H], F32R, tag=f"g_kk{i}",
                               name=f"g_kk{i}")
                    for i in range(6)]

            def load_g_k(g_d, i):
                o = (2 + i) * HT * H
                nc.sync.dma_start(out=g_kk[i][:], in_=g_d.ap()[:, o:o + HT * H])

            # ---- startup loads: >=4KB contiguous rows, consumption order ---
            nc.sync.dma_start(out=xa_t[:, 0:2 * BS], in_=xa_t_d.ap()[:, 0:2 * BS])
            nc.sync.dma_start(out=g_k0[:, 0:2 * H], in_=g_ap_d.ap()[:, 0:2 * H])
            nc.sync.dma_start(out=xa_t[:, 2 * BS:], in_=xa_t_d.ap()[:, 2 * BS:])
            nc.sync.dma_start(out=g_k0[:, 2 * H:], in_=g_ap_d.ap()[:, 2 * H:4 * H])
            nc.sync.dma_start(out=g_k1[:, 0:2 * H],
                              in_=g_ap_d.ap()[:, 4 * H:6 * H])
            nc.sync.dma_start(out=g_k1[:, 2 * H:],
                              in_=g_ap_d.ap()[:, 6 * H:8 * H])
            for i in range(4):
                load_g_k(g_ap_d, i)
            nc.sync.dma_start(out=xp_t[:], in_=xp_t_d.ap()[:])
            load_g_k(g_ap_d, 4)
            load_g_k(g_ap_d, 5)
            nc.sync.dma_start(out=xp_nat[:], in_=xp_nat_d.ap()[:])
            # (g_pa loads are emitted after br0-s1 so the WAR deps pick up
            #  br0's reads; xa_nat after those.)

            def g_lhsT(k, h, gt):
                if k == 0:
                    piece = g_k0
                elif k == 1:
                    piece = g_k1
                else:
                    piece = g_kk[k - 2]
                o = h * H + gt * P
                return piece[:, o:o + P]

            def xa_rhs(h):
                return xa_t[:, h * BS:(h + 1) * BS]

            def xa_lhsT(gi, b):
                o = gi * BS + b * S
                return xa_t[:, o:o + S]

            def xp_lhsT(gi, b):
                o = gi * BS + b * S
                return xp_t[:, o:o + S]

            # projT2[gt][g_part, k, b, s] — stage-1 output, stage-2 rhs.
            projT2 = [projpool.tile([P, K, BL, S], F32R, tag=f"projT2_{gt}",
                                    name=f"projT2_{gt}")
                      for gt in range(HT)]

            evac_state = [0]

            def evac(dst, src):
                # ping-pong PSUM evacuations between vector and scalar
                if evac_state[0] % 2 == 0:
                    nc.vector.tensor_copy(dst, src)
                else:
                    nc.scalar.copy(dst, src)
                evac_state[0] += 1

            # ---- PE warmup: get the p-state ramp going during DMA lead-in
            for w in range(6):
                acc = ps_acc.tile([P, BL, S], F32, tag="acc", name=f"warm{w}")
                nc.tensor.matmul(acc[:], wm[:, 0:P], wm[:],
                                 start=True, stop=True)

            def stage1(br):
                # k0 h-outer with 4 open accumulators: first matmuls need
                # only the first xa/g_k0 chunks.
                accs = [ps_acc.tile([P, BL, S], F32, tag="acc",
                                    name=f"s1a{br}k0g{gt}")
                        for gt in range(HT)]
                for h in range(HT):
                    for gt in range(HT):
                        nc.tensor.matmul(
                            accs[gt][:], g_lhsT(0, h, gt), xa_rhs(h),
                            start=(h == 0), stop=(h == HT - 1),
                            skip_group_check=True)
                for gt in range(HT):
                    evac(projT2[gt][:, 0, :, :], accs[gt][:])
                for k in range(1, K):
                    for gt in range(HT):
                        acc = ps_acc.tile([P, BL, S], F32, tag="acc",
                                          name=f"s1a{br}k{k}g{gt}")
                        for h in range(HT):
                            nc.tensor.matmul(
                                acc[:], g_lhsT(k, h, gt), xa_rhs(h),
                                start=(h == 0), stop=(h == HT - 1))
                        evac(projT2[gt][:, k, :, :], acc[:])
                    yield k

            # th_all[t_part, k, b, s]: tanh(stage-2) output, both branches
            # (WAR-recycled). Score ops slice [:, j, :, :] batched over b.
            th_all = work.tile([P, K, BL, S], F32, tag="th", bufs=1)

            def stage2_ck(br, lhsT_of, ck, bs=range(BL)):
                # raw[t, k, s] = tanh(sum_g right[t,g] proj[g,k,s]) per batch
                for b in bs:
                    acc2 = ps_acc.tile([P, KC, S], F32, tag="acc",
                                       name=f"s2a{br}b{b}c{ck}")
                    for gi in range(HT):
                        nc.tensor.matmul(
                            acc2[:],
                            lhsT_of(gi, b),
                            projT2[gi][:, ck * KC:(ck + 1) * KC, b, :],
                            start=(gi == 0), stop=(gi == HT - 1))
                    nc.scalar.activation(
                        th_all[:, ck * KC:(ck + 1) * KC, b, :], acc2[:], Tanh)

            def sca_all(vbc):
                # first-half score partial, batched over all 4 batches
                sca = work.tile([P, BL, S], F32, tag="sca", bufs=1)
                nc.vector.tensor_scalar_mul(sca[:], th_all[:, 0, :, :],
                                            vbc[:, 0:1])
                for j in range(1, KC):
                    nc.vector.scalar_tensor_tensor(
                        sca[:], th_all[:, j, :, :], vbc[:, j:j + 1], sca[:],
                        MULT, ADD)
                return sca

            def zout(br, b, e_t, nat_other, nat_self, out_d):
                # out = self + (E_T.T @ other) / Z, Z via ones-matmul.
                zp = ps_z.tile([P, 2], F32, tag="z", name=f"z{br}b{b}")
                nc.tensor.matmul(zp[:], e_t[:, b, :], ones_t[:],
                                 start=True, stop=True)
                rz = work.tile([P, 1], F32, tag="rz", bufs=4)
                nc.vector.reciprocal(rz[:], zp[:, 0:1])
                rp = ps_o.tile([P, H], F32, tag="o", name=f"o{br}b{b}")
                nc.tensor.matmul(rp[:], e_t[:, b, :], nat_other[:, b * H:(b + 1) * H],
                                 start=True, stop=True)
                ot = work.tile([P, H], F32, tag="ot", bufs=2)
                nc.vector.scalar_tensor_tensor(
                    ot[:], rp[:], rz[:, 0:1], nat_self[:, b * H:(b + 1) * H].bitcast(F32),
                    MULT, ADD)
                nc.sync.dma_start(out=out_d.ap()[b * P:(b + 1) * P, :],
                                  in_=ot[:])

            e_t0 = work.tile([P, BL, S], F32R, tag="e0", bufs=1)
            e_t1 = work.tile([P, BL, S], F32R, tag="e1", bufs=1)

            # ================= branch 0 (aspect) ========================
            for k in stage1(0):
                if k == 5:
                    stage2_ck(0, xp_lhsT, 0)
                    sca0 = sca_all(vbc_a)
            # br1 G prefetch (8KB rows); WAR-gated on br0-s1 reads
            nc.sync.dma_start(out=g_k0[:], in_=g_pa_d.ap()[:, 0:4 * H])
            nc.sync.dma_start(out=g_k1[:], in_=g_pa_d.ap()[:, 4 * H:8 * H])
            nc.sync.dma_start(out=xa_nat[:], in_=xa_nat_d.ap()[:])
            for i in range(6):
                load_g_k(g_pa_d, i)
            stage2_ck(0, xp_lhsT, 1)
            # batched second half + exp (runs under br1-s1)
            scb0 = work.tile([P, BL, S], F32, tag="scb", bufs=1)
            nc.vector.tensor_scalar_mul(scb0[:], th_all[:, KC, :, :],
                                        vbc_a[:, KC:KC + 1])
            for j in range(1, KC):
                nc.vector.scalar_tensor_tensor(
                    scb0[:], th_all[:, KC + j, :, :],
                    vbc_a[:, KC + j:KC + j + 1], scb0[:], MULT, ADD)
            nc.vector.tensor_tensor(scb0[:], sca0[:], scb0[:], ADD)
            nc.scalar.activation(e_t0[:], scb0[:], Exp)

            # ================= branch 1 (polarity) ======================
            # br1 stage 1 with br1-s2-ck0 spliced at k=3 and br0's z/out
            # matmuls interleaved so the PE never waits on softmax chains.
            zo = 0
            for k in stage1(1):
                if k == 5:
                    stage2_ck(1, xa_lhsT, 0)
                    sca1 = sca_all(vbc_p)
                elif k in (2, 3, 4, 6):
                    zout(0, zo, e_t0, xp_nat, xa_nat, out_a_d)
                    zo += 1

            # ---- tail: ck1 pair-at-a-time, 4-op vector chain + exp --------
            def chain_pair(pr):
                bs2 = slice(2 * pr, 2 * pr + 2)
                cha = work.tile([P, 2, S], F32, tag=f"cha{pr}", bufs=1)
                nc.vector.scalar_tensor_tensor(
                    cha[:], th_all[:, KC, bs2, :], vbc_p[:, KC:KC + 1],
                    sca1[:, bs2, :], MULT, ADD)
                for j in range(1, KC):
                    nc.vector.scalar_tensor_tensor(
                        cha[:], th_all[:, KC + j, bs2, :],
                        vbc_p[:, KC + j:KC + j + 1], cha[:], MULT, ADD)
                nc.scalar.activation(e_t1[:, bs2, :], cha[:], Exp)

            def ztail(b, on_vector):
                zp = ps_z.tile([P, 2], F32, tag="z", name=f"z1b{b}")
                nc.tensor.matmul(zp[:], e_t1[:, b, :], ones_t[:],
                                 start=True, stop=True)
                rz = work.tile([P, 1], F32, tag="rz", bufs=4)
                nc.vector.reciprocal(rz[:], zp[:, 0:1])
                rp = ps_o.tile([P, H], F32, tag="o", name=f"o1b{b}")
                nc.tensor.matmul(rp[:], e_t1[:, b, :], xa_nat[:, b * H:(b + 1) * H],
                                 start=True, stop=True)
                if on_vector:
                    otf = work.tile([P, H], F32, tag="otf_v", bufs=2)
                    nc.vector.scalar_tensor_tensor(
                        otf[:], rp[:], rz[:, 0:1],
                        xp_nat[:, b * H:(b + 1) * H].bitcast(F32), MULT, ADD)
                else:
                    # vector is busy with the other pair's chain: combine on
                    # scalar (act-Copy-scale) + gpsimd (tensor add)
                    ots = work.tile([P, H], F32, tag="ots", bufs=2)
                    nc.scalar.activation(ots[:], rp[:], Copy, scale=rz[:, 0:1])
                    otf = work.tile([P, H], F32, tag="otf", bufs=2)
                    nc.gpsimd.tensor_tensor(
                        otf[:], ots[:],
                        xp_nat[:, b * H:(b + 1) * H].bitcast(F32), ADD)
                nc.sync.dma_start(out=out_p_d.ap()[b * P:(b + 1) * P, :],
                                  in_=otf[:])

            stage2_ck(1, xa_lhsT, 1, bs=(0, 1))
            chain_pair(0)                     # under b2/b3 matmuls
            stage2_ck(1, xa_lhsT, 1, bs=(2, 3))
            ztail(0, on_vector=False)
            ztail(1, on_vector=False)
            chain_pair(1)                     # under b0/b1 z/out matmuls
            ztail(2, on_vector=True)
            ztail(3, on_vector=True)

    nc.compile()
    return nc


def _get_nc():
    if "nc" not in _cache:
        _cache["nc"] = _build()
    return _cache["nc"]


def _prep_in_maps(aspect_hidden, polarity_hidden, G_aspect_polarity,
                  G_polarity_aspect, G_vector_aspect, G_vector_polarity):
    f = np.float32

    def shuffle_g(g):
        # host-side image of the SBUF G tiles, concatenated in consumption
        # order: k0 h-major block, k1 h-major block, then per-k blocks
        gr = np.asarray(g, dtype=f).reshape(HT, P, K, H)
        pieces = [gr[:, :, k, :].transpose(1, 0, 2).reshape(P, HT * H)
                  for k in range(K)]
        return np.ascontiguousarray(np.concatenate(pieces, axis=1))

    def shuffle_t(x_loc):
        # [BS,H] -> transposed partition-major [P, (ht, bs)]
        return np.ascontiguousarray(
            x_loc.T.reshape(HT, P, BS).transpose(1, 0, 2))

    def shuffle_nat(x_loc):
        # [BS,H] -> partition-major [P, (b, h)]
        return np.ascontiguousarray(
            x_loc.reshape(BL, P, H).transpose(1, 0, 2).reshape(P, BL * H))

    a = np.ascontiguousarray(aspect_hidden, dtype=f)
    p = np.ascontiguousarray(polarity_hidden, dtype=f)
    g_ap = shuffle_g(G_aspect_polarity)
    g_pa = shuffle_g(G_polarity_aspect)
    v_ap = np.ascontiguousarray(G_vector_aspect, dtype=f)
    v_pa = np.ascontiguousarray(G_vector_polarity, dtype=f)

    in_maps = []
    for c in range(NCORES):
        a_loc = a[c * BL:(c + 1) * BL].reshape(BS, H)
        p_loc = p[c * BL:(c + 1) * BL].reshape(BS, H)
        m = {
            "xa_t": np.ascontiguousarray(shuffle_t(a_loc).reshape(P, HT * BS)),
            "xp_t": np.ascontiguousarray(shuffle_t(p_loc).reshape(P, HT * BS)),
            "xa_nat": shuffle_nat(a_loc),
            "xp_nat": shuffle_nat(p_loc),
            "g_ap": g_ap,
            "g_pa": g_pa,
            "v_ap": v_ap,
            "v_pa": v_pa,
        }
        in_maps.append(m)
    return in_maps


def kernel(aspect_hidden, polarity_hidden, G_aspect_polarity,
           G_polarity_aspect, G_vector_aspect, G_vector_polarity):
    nc = _get_nc()
    in_maps = _prep_in_maps(aspect_hidden, polarity_hidden, G_aspect_polarity,
                            G_polarity_aspect, G_vector_aspect,
                            G_vector_polarity)
    res = run_bass_kernel_spmd(
        nc, in_maps, core_ids=list(range(NCORES)),
        trace=bool(os.environ.get("KERNEL_TRACE")))
    _cache["last_results"] = res

    out_a = np.empty((B, S, H), np.float32)
    out_p = np.empty((B, S, H), np.float32)
    for c in range(NCORES):
        out_a[c * BL:(c + 1) * BL] = res.results[c]["out_a"].reshape(BL, S, H)
        out_p[c * BL:(c + 1) * BL] = res.results[c]["out_p"].reshape(BL, S, H)
    return (out_a, out_p)


# revision 7
# speedup vs baseline: 1.0986x; 1.0959x over previous
"""CrossSharedUnit Trainium2 kernel — 8-core data-parallel over batch.

Reference computation (per batch b, S=128 tokens, H=512 hidden, K=8):
  proj[b,s,k,g] = sum_h left[b,s,h] * G[h,k,g]
  raw[b,s,t,k]  = tanh(sum_g proj[b,s,k,g] * right[b,t,g])
  score[b,s,t]  = sum_k raw[b,s,t,k] * v[k]
  attn          = softmax(score, axis=t)
  out           = self + attn @ other_hidden
for two branches (aspect: left=aspect, right=polarity; polarity: left=aspect,
right=aspect — faithful to the source which uses aspect on BOTH sides).

Sharding: batch B=32 split 4-per-core across 8 cores; G tensors replicated.
No collectives.

Precision: all matmul operands are fp16 with fp32 PSUM accumulation — fp16's
10-bit mantissa matches the fp32r (TF32) datapath the fp32 version would use,
so accuracy is unchanged (measured ~2e-3 rel err, gate 2e-2) while every
input transfer halves and the DVE score chains run in 2x 16-bit mode.
exp() cannot overflow fp16: |score| <= sum|v_k| ~ 7.7 -> e^7.7 ~ 2.2e3.

Schedule: the PE (tensor engine) is the bottleneck (~76us of matmul), so the
program is one continuous PE stream:
  warmup | br0-s1 (+ br0-s2-ck0 spliced at k=5) | br0-s2-ck1
         | br1-s1 (+ br1-s2-ck0 splice + br0 z/out mms interleaved)
         | br1-s2-ck1 (pairwise, z/out + wm fillers interleaved)
with softmax chains on vector/scalar underneath the next phase's matmuls.

DMA: all input loads ride the sync queue in exact consumption order. The
startup-critical tensors (xa + G k0 block) are packed interleaved in two
dedicated head images so each dma_start covers >=4KB-contiguous partition
rows (small descriptors halve early DMA throughput), letting the first real
matmul start right as the warmup ramp ends.

The softmax division is deferred through the attention matmul:
out = self + (E @ other) / Z with Z from a ones-matmul.
"""

import os
import sys

sys.path.insert(0, "/opt/trn_rl_repo")

import numpy as np

from concourse import bacc, mybir, tile
from concourse.bass_utils import run_bass_kernel_spmd

B, S, H, K = 32, 128, 512, 8
NCORES = 8
BL = B // NCORES          # batches per core
BS = BL * S               # rows per core (512)
P = 128                   # partitions
HT = H // P               # h partition-tiles (4)
KG = K * H                # flattened (k,g) axis (4096)
KC = K // 2               # k's per stage-2 chunk (4)
F32 = mybir.dt.float32
F32R = mybir.dt.float32r
F16 = mybir.dt.float16

_cache = {}


def _build():
    """Build + compile the per-core Bass program (same program on all cores)."""
    nc = bacc.Bacc("TRN2", target_bir_lowering=False, debug=False,
                   num_devices=NCORES)

    # head images: [xa_h0|xa_h1|g_k0h0|g_k0h1] and the h2/h3 twin (fp16)
    head_a_d = nc.dram_tensor("head_a", [P, 2 * BS + 2 * H], F16,
                              kind="ExternalInput")
    head_b_d = nc.dram_tensor("head_b", [P, 2 * BS + 2 * H], F16,
                              kind="ExternalInput")
    xp_t_d = nc.dram_tensor("xp_t", [P, HT * BS], F16, kind="ExternalInput")
    xa_nat_d = nc.dram_tensor("xa_nat", [P, BL * H], F16, kind="ExternalInput")
    xp_nat_d = nc.dram_tensor("xp_nat", [P, BL * H], F16, kind="ExternalInput")
    # G: one pre-shuffled tensor per branch; column blocks in consumption
    # order: k0 h-major block (4H), k1 h-major block (4H), then per-k blocks
    g_ap_d = nc.dram_tensor("g_ap", [P, HT * KG], F16, kind="ExternalInput")
    g_pa_d = nc.dram_tensor("g_pa", [P, HT * KG], F16, kind="ExternalInput")
    v_ap_d = nc.dram_tensor("v_ap", [K, 1], F32, kind="ExternalInput")
    v_pa_d = nc.dram_tensor("v_pa", [K, 1], F32, kind="ExternalInput")
    out_a_d = nc.dram_tensor("out_a", [BS, H], F32, kind="ExternalOutput")
    out_p_d = nc.dram_tensor("out_p", [BS, H], F32, kind="ExternalOutput")

    Tanh = mybir.ActivationFunctionType.Tanh
    Exp = mybir.ActivationFunctionType.Exp
    MULT = mybir.AluOpType.mult
    ADD = mybir.AluOpType.add

    with tile.TileContext(nc) as tc:
        with (
            tc.tile_pool(name="const", bufs=1) as cpool,
            tc.tile_pool(name="g", bufs=1) as gpool,
            tc.tile_pool(name="proj", bufs=1) as projpool,
            tc.tile_pool(name="work", bufs=2) as work,
            tc.tile_pool(name="ps_acc", bufs=4, space="PSUM") as ps_acc,
            tc.tile_pool(name="ps_o", bufs=2, space="PSUM") as ps_o,
            tc.tile_pool(name="ps_z", bufs=2, space="PSUM") as ps_z,
        ):
            # ---- constants + warmup weights --------------------------------
            wm = cpool.tile([P, BS], F32R, tag="wm")
            nc.vector.memset(wm[:].bitcast(F32), 0.0)
            ones_t = cpool.tile([P, 2], F16, tag="ones_t")
            nc.vector.memset(ones_t[:], 1.0)

            # ---- persistent activations ------------------------------------
            head_a = cpool.tile([P, 2 * BS + 2 * H], F16, tag="head_a")
            head_b = cpool.tile([P, 2 * BS + 2 * H], F16, tag="head_b")
            xp_t = cpool.tile([P, HT * BS], F16, tag="xp_t")
            xa_nat = cpool.tile([P, BL * H], F16, tag="xa_nat")
            xp_nat = cpool.tile([P, BL * H], F16, tag="xp_nat")

            vrow_a = cpool.tile([1, K], F32, tag="vrow_a")
            vrow_p = cpool.tile([1, K], F32, tag="vrow_p")
            nc.gpsimd.dma_start(out=vrow_a[:], in_=v_ap_d.ap().rearrange("k o -> o k"))
            nc.gpsimd.dma_start(out=vrow_p[:], in_=v_pa_d.ap().rearrange("k o -> o k"))
            vbc_a = cpool.tile([P, K], F32, tag="vbc_a")
            vbc_p = cpool.tile([P, K], F32, tag="vbc_p")
            nc.gpsimd.partition_broadcast(vbc_a[:], vrow_a[:])
            nc.gpsimd.partition_broadcast(vbc_p[:], vrow_p[:])

            # ---- G piece tiles (bufs=1; k0/k1 recycled for branch 1) -------
            g_k0 = gpool.tile([P, HT * H], F16, tag="g_k0")
            g_k1 = gpool.tile([P, HT * H], F16, tag="g_k1")
            g_kk = [gpool.tile([P, HT * H], F16, tag=f"g_kk{i}",
                               name=f"g_kk{i}")
                    for i in range(6)]

            def load_g_k(g_d, i):
                o = (2 + i) * HT * H
                nc.sync.dma_start(out=g_kk[i][:], in_=g_d.ap()[:, o:o + HT * H])

            # ---- startup loads, consumption order --------------------------
            nc.sync.dma_start(out=head_a[:], in_=head_a_d.ap()[:])
            nc.sync.dma_start(out=head_b[:], in_=head_b_d.ap()[:])
            nc.sync.dma_start(out=g_k1[:], in_=g_ap_d.ap()[:, 4 * H:8 * H])
            for i in range(4):
                load_g_k(g_ap_d, i)
            nc.sync.dma_start(out=xp_t[:], in_=xp_t_d.ap()[:])
            load_g_k(g_ap_d, 4)
            load_g_k(g_ap_d, 5)
            nc.sync.dma_start(out=xp_nat[:], in_=xp_nat_d.ap()[:])
            # (g_pa loads are emitted after br0-s1 so the WAR deps pick up
            #  br0's reads; xa_nat after those.)

            def xa_rhs(h):
                t = head_a if h < 2 else head_b
                return t[:, (h % 2) * BS:(h % 2 + 1) * BS]

            def xa_lhsT(gi, b):
                t = head_a if gi < 2 else head_b
                o = (gi % 2) * BS + b * S
                return t[:, o:o + S]

            def xp_lhsT(gi, b):
                o = gi * BS + b * S
                return xp_t[:, o:o + S]

            def g0_head(h, gt):
                # branch-0 k0 weights live in the head images
                t = head_a if h < 2 else head_b
                o = 2 * BS + (h % 2) * H + gt * P
                return t[:, o:o + P]

            def g0_tile(h, gt):
                return g_k0[:, h * H + gt * P:h * H + gt * P + P]

            def g_lhsT(k, h, gt, g0_at):
                if k == 0:
                    return g0_at(h, gt)
                piece = g_k1 if k == 1 else g_kk[k - 2]
                o = h * H + gt * P
                return piece[:, o:o + P]

            # projT2[gt][g_part, k, b, s] — stage-1 output, stage-2 rhs.
            projT2 = [projpool.tile([P, K, BL, S], F16, tag=f"projT2_{gt}",
                                    name=f"projT2_{gt}")
                      for gt in range(HT)]

            evac_state = [0]

            def evac(dst, src):
                # ping-pong PSUM evacuations between vector and scalar
                if evac_state[0] % 2 == 0:
                    nc.vector.tensor_copy(dst, src)
                else:
                    nc.scalar.copy(dst, src)
                evac_state[0] += 1

            # ---- PE warmup: get the p-state ramp going during DMA lead-in
            for w in range(7):
                acc = ps_acc.tile([P, BL, S], F32, tag="acc", name=f"warm{w}")
                nc.tensor.matmul(acc[:], wm[:, 0:P], wm[:],
                                 start=True, stop=True)

            def filler(name):
                acc = ps_acc.tile([P, BL, S], F32, tag="acc", name=name)
                nc.tensor.matmul(acc[:], wm[:, 0:P], wm[:],
                                 start=True, stop=True)

            def stage1(br, g0_at):
                # k0 h-outer with 4 open accumulators: first matmuls need
                # only the head_a image.
                accs = [ps_acc.tile([P, BL, S], F32, tag="acc",
                                    name=f"s1a{br}k0g{gt}")
                        for gt in range(HT)]
                for h in range(HT):
                    for gt in range(HT):
                        nc.tensor.matmul(
                            accs[gt][:], g_lhsT(0, h, gt, g0_at), xa_rhs(h),
                            start=(h == 0), stop=(h == HT - 1),
                            skip_group_check=True)
                for gt in range(HT):
                    evac(projT2[gt][:, 0, :, :], accs[gt][:])
                for k in range(1, K):
                    for gt in range(HT):
                        acc = ps_acc.tile([P, BL, S], F32, tag="acc",
                                          name=f"s1a{br}k{k}g{gt}")
                        for h in range(HT):
                            nc.tensor.matmul(
                                acc[:], g_lhsT(k, h, gt, g0_at), xa_rhs(h),
                                start=(h == 0), stop=(h == HT - 1))
                        evac(projT2[gt][:, k, :, :], acc[:])
                    yield k

            # th_all[t_part, k, b, s]: tanh(stage-2) output, both branches
            # (WAR-recycled). Score ops slice [:, j, :, :] batched over b.
            th_all = work.tile([P, K, BL, S], F16, tag="th", bufs=1)

            def stage2_ck(br, lhsT_of, ck, bs=range(BL)):
                # raw[t, k, s] = tanh(sum_g right[t,g] proj[g,k,s]) per batch
                for b in bs:
                    acc2 = ps_acc.tile([P, KC, S], F32, tag="acc",
                                       name=f"s2a{br}b{b}c{ck}")
                    for gi in range(HT):
                        nc.tensor.matmul(
                            acc2[:],
                            lhsT_of(gi, b),
                            projT2[gi][:, ck * KC:(ck + 1) * KC, b, :],
                            start=(gi == 0), stop=(gi == HT - 1))
                    nc.scalar.activation(
                        th_all[:, ck * KC:(ck + 1) * KC, b, :], acc2[:], Tanh)

            def sca_all(vbc):
                # first-half score partial, batched over all 4 batches
                sca = work.tile([P, BL, S], F16, tag="sca", bufs=1)
                nc.vector.tensor_scalar_mul(sca[:], th_all[:, 0, :, :],
                                            vbc[:, 0:1])
                for j in range(1, KC):
                    nc.vector.scalar_tensor_tensor(
                        sca[:], th_all[:, j, :, :], vbc[:, j:j + 1], sca[:],
                        MULT, ADD)
                return sca

            def zout(br, b, e_t, nat_other, nat_self, out_d):
                # out = self + (E_T.T @ other) / Z, Z via ones-matmul.
                zp = ps_z.tile([P, 2], F32, tag="z", name=f"z{br}b{b}")
                nc.tensor.matmul(zp[:], e_t[:, b, :], ones_t[:],
                                 start=True, stop=True)
                rz = work.tile([P, 1], F32, tag="rz", bufs=4)
                nc.vector.reciprocal(rz[:], zp[:, 0:1])
                rp = ps_o.tile([P, H], F32, tag="o", name=f"o{br}b{b}")
                nc.tensor.matmul(rp[:], e_t[:, b, :], nat_other[:, b * H:(b + 1) * H],
                                 start=True, stop=True)
                ot = work.tile([P, H], F32, tag="ot", bufs=2)
                nc.vector.scalar_tensor_tensor(
                    ot[:], rp[:], rz[:, 0:1], nat_self[:, b * H:(b + 1) * H],
                    MULT, ADD)
                nc.sync.dma_start(out=out_d.ap()[b * P:(b + 1) * P, :],
                                  in_=ot[:])

            e_t0 = work.tile([P, BL, S], F16, tag="e0", bufs=1)
            e_t1 = work.tile([P, BL, S], F16, tag="e1", bufs=1)

            # ================= branch 0 (aspect) ========================
            for k in stage1(0, g0_head):
                if k == 5:
                    stage2_ck(0, xp_lhsT, 0)
                    sca0 = sca_all(vbc_a)
            # br1 G prefetch; WAR-gated on br0-s1 reads
            nc.sync.dma_start(out=g_k0[:], in_=g_pa_d.ap()[:, 0:4 * H])
            nc.sync.dma_start(out=g_k1[:], in_=g_pa_d.ap()[:, 4 * H:8 * H])
            nc.sync.dma_start(out=xa_nat[:], in_=xa_nat_d.ap()[:])
            for i in range(6):
                load_g_k(g_pa_d, i)
            stage2_ck(0, xp_lhsT, 1)
            # batched second half + exp (runs under br1-s1)
            scb0 = work.tile([P, BL, S], F16, tag="scb", bufs=1)
            nc.vector.tensor_scalar_mul(scb0[:], th_all[:, KC, :, :],
                                        vbc_a[:, KC:KC + 1])
            for j in range(1, KC):
                nc.vector.scalar_tensor_tensor(
                    scb0[:], th_all[:, KC + j, :, :],
                    vbc_a[:, KC + j:KC + j + 1], scb0[:], MULT, ADD)
            nc.vector.tensor_tensor(scb0[:], sca0[:], scb0[:], ADD)
            nc.scalar.activation(e_t0[:], scb0[:], Exp)

            # ================= branch 1 (polarity) ======================
            # br1 stage 1 with br1-s2-ck0 spliced at k=5 and br0's z/out
            # matmuls interleaved so the PE never waits on softmax chains.
            zo = 0
            for k in stage1(1, g0_tile):
                if k == 5:
                    stage2_ck(1, xa_lhsT, 0)
                    sca1 = sca_all(vbc_p)
                elif k in (2, 3, 4, 6):
                    zout(0, zo, e_t0, xp_nat, xa_nat, out_a_d)
                    zo += 1

            # ---- tail: ck1 pair-at-a-time, 4-op fp16 vector chain + exp ---
            def chain_pair(pr):
                bs2 = slice(2 * pr, 2 * pr + 2)
                cha = work.tile([P, 2, S], F16, tag=f"cha{pr}", bufs=1)
                nc.vector.scalar_tensor_tensor(
                    cha[:], th_all[:, KC, bs2, :], vbc_p[:, KC:KC + 1],
                    sca1[:, bs2, :], MULT, ADD)
                for j in range(1, KC):
                    nc.vector.scalar_tensor_tensor(
                        cha[:], th_all[:, KC + j, bs2, :],
                        vbc_p[:, KC + j:KC + j + 1], cha[:], MULT, ADD)
                nc.scalar.activation(e_t1[:, bs2, :], cha[:], Exp)

            def ztail(b):
                zp = ps_z.tile([P, 2], F32, tag="z", name=f"z1b{b}")
                nc.tensor.matmul(zp[:], e_t1[:, b, :], ones_t[:],
                                 start=True, stop=True)
                rz = work.tile([P, 1], F32, tag="rz", bufs=4)
                nc.vector.reciprocal(rz[:], zp[:, 0:1])
                rp = ps_o.tile([P, H], F32, tag="o", name=f"o1b{b}")
                nc.tensor.matmul(rp[:], e_t1[:, b, :], xa_nat[:, b * H:(b + 1) * H],
                                 start=True, stop=True)
                otf = work.tile([P, H], F32, tag="otf", bufs=2)
                nc.vector.scalar_tensor_tensor(
                    otf[:], rp[:], rz[:, 0:1], xp_nat[:, b * H:(b + 1) * H],
                    MULT, ADD)
                nc.sync.dma_start(out=out_p_d.ap()[b * P:(b + 1) * P, :],
                                  in_=otf[:])

            stage2_ck(1, xa_lhsT, 1, bs=(0, 1))
            chain_pair(0)                     # under b2/b3 matmuls
            stage2_ck(1, xa_lhsT, 1, bs=(2, 3))
            chain_pair(1)                     # vector, right after tanh b3
            filler("f0")                      # keep the PE clock up while
            ztail(0)                          # exp01 lands
            ztail(1)
            filler("f1")
            filler("f2")
            ztail(2)
            ztail(3)

    nc.compile()
    return nc


def _get_nc():
    if "nc" not in _cache:
        _cache["nc"] = _build()
    return _cache["nc"]


def _prep_in_maps(aspect_hidden, polarity_hidden, G_aspect_polarity,
                  G_polarity_aspect, G_vector_aspect, G_vector_polarity):
    f16 = np.float16

    def shuffle_g(g):
        # host-side image of the SBUF G tiles, concatenated in consumption
        # order: k0 h-major block, k1 h-major block, then per-k blocks
        gr = np.asarray(g, f16).reshape(HT, P, K, H)
        pieces = [gr[:, :, k, :].transpose(1, 0, 2).reshape(P, HT * H)
                  for k in range(K)]
        return np.ascontiguousarray(np.concatenate(pieces, axis=1))

    def shuffle_t(x_loc):
        # [BS,H] -> transposed partition-major [P, (ht, bs)]
        return np.ascontiguousarray(
            x_loc.T.reshape(HT, P, BS).transpose(1, 0, 2).reshape(P, HT * BS))

    def shuffle_nat(x_loc):
        # [BS,H] -> partition-major [P, (b, h)]
        return np.ascontiguousarray(
            x_loc.reshape(BL, P, H).transpose(1, 0, 2).reshape(P, BL * H))

    a = np.asarray(aspect_hidden, f16)
    p = np.asarray(polarity_hidden, f16)
    g_ap = shuffle_g(G_aspect_polarity)
    g_pa = shuffle_g(G_polarity_aspect)
    v_ap = np.ascontiguousarray(G_vector_aspect, np.float32)
    v_pa = np.ascontiguousarray(G_vector_polarity, np.float32)

    in_maps = []
    for c in range(NCORES):
        a_loc = a[c * BL:(c + 1) * BL].reshape(BS, H)
        p_loc = p[c * BL:(c + 1) * BL].reshape(BS, H)
        xa_t = shuffle_t(a_loc)
        m = {
            "head_a": np.ascontiguousarray(
                np.concatenate([xa_t[:, 0:2 * BS], g_ap[:, 0:2 * H]], axis=1)),
            "head_b": np.ascontiguousarray(
                np.concatenate([xa_t[:, 2 * BS:], g_ap[:, 2 * H:4 * H]], axis=1)),
            "xp_t": shuffle_t(p_loc),
            "xa_nat": shuffle_nat(a_loc),
            "xp_nat": shuffle_nat(p_loc),
            "g_ap": g_ap,
            "g_pa": g_pa,
            "v_ap": v_ap,
            "v_pa": v_pa,
        }
        in_maps.append(m)
    return in_maps


def kernel(aspect_hidden, polarity_hidden, G_aspect_polarity,
           G_polarity_aspect, G_vector_aspect, G_vector_polarity):
    nc = _get_nc()
    in_maps = _prep_in_maps(aspect_hidden, polarity_hidden, G_aspect_polarity,
                            G_polarity_aspect, G_vector_aspect,
                            G_vector_polarity)
    res = run_bass_kernel_spmd(
        nc, in_maps, core_ids=list(range(NCORES)),
        trace=bool(os.environ.get("KERNEL_TRACE")))
    _cache["last_results"] = res

    out_a = np.empty((B, S, H), np.float32)
    out_p = np.empty((B, S, H), np.float32)
    for c in range(NCORES):
        out_a[c * BL:(c + 1) * BL] = res.results[c]["out_a"].reshape(BL, S, H)
        out_p[c * BL:(c + 1) * BL] = res.results[c]["out_p"].reshape(BL, S, H)
    return (out_a, out_p)
